# revision 35
# baseline (speedup 1.0000x reference)
"""Sparse attention (per-query top-K) Trainium2 kernel, 8-core tensor-parallel.

Strategy (heads sharded 2-per-core, dense-score formulation):
  - Host folds idx/valid/geo_bias into per-(s,q) merged bias factors
    E[s,q] = sum_{j: idx[q,j]==s} exp(geo_bias[h,q,j]), stored as causal
    fp16 tiles.  This turns the per-query gather/softmax into dense math:
        A^T = E^T * exp(S^T - C),   S^T = K @ Q^T (feature-major)
        out^T = [V | 1]^T @ A^T     (row 64 = softmax denominator)
  - Single software-pipelined main loop: the projection matmuls for
    query-tile t+1 (bf16, x pre-transposed on host) are interleaved into
    the attention chunk loop of tile t so the PE fills the slack while
    ACT (the exp bottleneck) streams.  The two heads' 64-contraction
    score matmuls run concurrently in row-halves of the PE array; exp on
    ACT covers both heads per instruction; the E-multiply runs on DVE
    once per 4 chunks (fp16 2x mode); AV on PE in fp16.
  - AllToAll reshards the (unnormalized) head outputs + denominators so
    each core owns 512 query rows with all 16 heads, normalizes, and
    computes its o_proj row-slice with the full Wo in bf16.  Host
    concatenates.
"""

import sys

sys.path.insert(0, "/opt/trn_rl_repo")

import numpy as np
import ml_dtypes

from concourse import bacc, mybir, tile
from concourse.bass_utils import run_bass_kernel_spmd
from concourse.masks import make_identity

F32 = mybir.dt.float32
F16 = mybir.dt.float16
BF16 = mybir.dt.bfloat16

S = 4096
H = 1024
NH = 16
KSEL = 32
HD = 64
NC = 8
HPC = NH // NC  # 2 heads per core
QT = 512
NQT = S // QT
SC = 128
CSHIFT = 2.0
SLAB = 8  # s-chunks per E-tile DMA slab
QUAD = 2  # chunks per DVE multiply

TILE_LIST = [(t, c) for t in range(NQT) for c in range(4 * (t + 1))]
N_TILES = len(TILE_LIST)  # 144
TILE_IDX = {tc: n for n, tc in enumerate(TILE_LIST)}


def _build_program(phases=3, n_cores_build=NC):
    nc = bacc.Bacc(
        "TRN2", target_bir_lowering=False, debug=False, num_devices=n_cores_build
    )

    # host-pretransposed x: [st, p, hc, 512] (feature-major, bf16)
    xT_in = nc.dram_tensor("xT", [NQT, 128, 8, QT], BF16, kind="ExternalInput").ap()
    wq_in = nc.dram_tensor("wq", [128, 8, 128], BF16, kind="ExternalInput").ap()
    wk_in = nc.dram_tensor("wk", [128, 8, 128], BF16, kind="ExternalInput").ap()
    wv_in = nc.dram_tensor("wv", [128, 8, 128], BF16, kind="ExternalInput").ap()
    wo_in = nc.dram_tensor("wo", [128, 8, H], BF16, kind="ExternalInput").ap()
    bo_in = nc.dram_tensor("bo_rep", [128, H], F32, kind="ExternalInput").ap()
    e_in = nc.dram_tensor(
        "e_pack", [N_TILES, SC, HPC, QT], F16, kind="ExternalInput"
    ).ap()
    sel_in = nc.dram_tensor("sel16", [NH, H], F32, kind="ExternalInput").ap()
    y_out = nc.dram_tensor("y_part", [QT, H], F32, kind="ExternalOutput").ap()

    with tile.TileContext(nc) as tc:
        with (
            tc.tile_pool(name="const", bufs=1) as constp,
            tc.tile_pool(name="persist", bufs=1) as persist,
            tc.tile_pool(name="dram", bufs=1, space="DRAM") as dram,
        ):
            ident = constp.tile([128, 128], F32, tag="ident")
            make_identity(nc, ident[:])
            nbias = constp.tile([128, 1], F32, tag="nbias")
            nc.gpsimd.memset(nbias[:], -CSHIFT)

            wq_sb = constp.tile([128, 8, 128], BF16, tag="wq")
            wk_sb = constp.tile([128, 8, 128], BF16, tag="wk")
            wv_sb = constp.tile([128, 8, 128], BF16, tag="wv")
            # wo/bo/sel are loaded later (mid phase 2) to keep startup lean
            wo_sb = constp.tile([128, 8, H], BF16, tag="wo")
            bo_sb = constp.tile([128, H], F32, tag="bo")
            sel_sb = constp.tile([NH, H], F32, tag="sel")

            qT_sb = persist.tile([128, NQT, QT], BF16, tag="qT")
            kT_sb = persist.tile([128, NQT, QT], BF16, tag="kT")
            # wave-0 phase-3 inputs live in the persistent pool so their
            # loads can be issued from inside the main loop
            den0_sb = persist.tile([NH, 256], F16, tag="den0")
            rden0_sb = persist.tile([NH, 256], F32, tag="rden0")
            oT0_sb = persist.tile([128, 8, 256], F16, tag="oT0")
            v_sb = [
                persist.tile([128, S // SC, HD + 1], F16, tag=f"v{h}", name=f"v{h}")
                for h in range(HPC)
            ]
            for h in range(HPC):
                # only the ones-column matters; cols 0:64 are overwritten
                nc.gpsimd.memset(v_sb[h][:, :, HD : HD + 1], 1.0)

            # two AllToAll waves of 4 q-tiles each: wave 0's collective
            # blocks the gpsimd queue only ~30us (absorbed by the e-slab
            # prefetch ring); wave 1 runs on a quiet device at the end
            WTILES = [list(range(0, 4)), list(range(4, 8))]
            WQ = [len(ws) * QT // NC for ws in WTILES]  # 256, 256
            a2a_in = [
                dram.tile(
                    [NC, HPC * (HD + 1), WQ[w]], F16, tag=f"ain{w}", name=f"ain{w}"
                )
                for w in range(2)
            ]
            a2a_out = [
                dram.tile(
                    [NC, HPC * (HD + 1), WQ[w]], F16, tag=f"aout{w}", name=f"aout{w}"
                )
                for w in range(2)
            ]

            main_pools = (
                tc.tile_pool(name="xT", bufs=3),
                tc.tile_pool(name="vtmp", bufs=2),
                tc.tile_pool(name="zap", bufs=3),
                tc.tile_pool(name="ep", bufs=5),
                tc.tile_pool(name="otp", bufs=2),
                tc.tile_pool(name="p2s", bufs=2, space="PSUM"),
                tc.tile_pool(name="p2o", bufs=1, space="PSUM"),
                tc.tile_pool(name="p1ps", bufs=1, space="PSUM"),
            )
            xTp, vtmpp, zap, epool, otp, p2s, p2o, p1ps = (
                p.__enter__() for p in main_pools
            )
            xT_tiles = {}

            dma_no = [0]

            def big_dma(dst, src):
                eng = (nc.sync, nc.gpsimd)[dma_no[0] % 2]
                eng.dma_start(dst, src)
                dma_no[0] += 1

            def load_xT(st, split=False):
                xt = xTp.tile([128, 8, QT], BF16, tag="xT", name=f"xT{st}")
                xT_tiles[st] = xt
                if split:  # per-hc chunks so the first matmul starts ASAP
                    for hc in range(8):
                        nc.sync.dma_start(xt[:, hc, :], xT_in[st, :, hc, :])
                    dma_no[0] += 1
                else:
                    big_dma(xt[:], xT_in[st])

            # ---- projection work for one s-tile, as two piece lists ----
            def proj_pieces(st):
                pieces = []
                state = {}

                def mk_mm(w_sb, crange, tag, fin):
                    def go():
                        if tag not in state:
                            state[tag] = p1ps.tile([128, QT], F32, tag="proj", name=f"proj_{tag}")
                        ps = state[tag]
                        for c in crange:
                            nc.tensor.matmul(
                                ps[:], w_sb[:, c, :], xT_tiles[st][:, c, :],
                                start=(c == 0), stop=(c == 7),
                            )
                        if fin is not None:
                            fin(ps)

                    return go

                def fin_q(ps):
                    nc.vector.tensor_copy(qT_sb[:, st, :], ps[:])

                def fin_k(ps):
                    nc.vector.tensor_copy(kT_sb[:, st, :], ps[:])

                def fin_v(ps):
                    vt = vtmpp.tile([128, QT], F32, tag="vt", name="vt")
                    state["vt"] = vt
                    nc.vector.tensor_copy(vt[:], ps[:])

                qk = [
                    mk_mm(wq_sb, range(0, 4), "q", None),
                    mk_mm(wq_sb, range(4, 8), "q", fin_q),
                    mk_mm(wk_sb, range(0, 4), "k", None),
                    mk_mm(wk_sb, range(4, 8), "k", fin_k),
                ]
                pieces.append(mk_mm(wv_sb, range(0, 4), "v", None))
                pieces.append(mk_mm(wv_sb, range(4, 8), "v", fin_v))

                def transpose_v():
                    vt = state["vt"]
                    ps_tv = p1ps.tile([128, QT], F32, tag="tp")
                    for i in range(4):
                        nc.tensor.transpose(
                            ps_tv[:, i * 128 : (i + 1) * 128],
                            vt[:, i * 128 : (i + 1) * 128],
                            ident[:],
                        )
                    ps_tv4 = ps_tv[:].rearrange("p (i h d) -> p i h d", i=4, h=HPC)
                    for h in range(HPC):
                        nc.vector.tensor_copy(
                            v_sb[h][:, st * 4 : (st + 1) * 4, 0:HD],
                            ps_tv4[:, :, h, :],
                        )

                pieces.append(transpose_v)
                return qk, pieces

            # ---- main software-pipelined loop ---------------------------
            # first-needed bytes first: xT0 low half, wq, xT0 high, wk, wv
            xt0 = xTp.tile([128, 8, QT], BF16, tag="xT", name="xT0")
            xT_tiles[0] = xt0
            for hc in range(4):
                nc.sync.dma_start(xt0[:, hc, :], xT_in[0, :, hc, :])
            nc.sync.dma_start(wq_sb[:], wq_in[:])
            for hc in range(4, 8):
                nc.sync.dma_start(xt0[:, hc, :], xT_in[0, :, hc, :])
            nc.sync.dma_start(wk_sb[:], wk_in[:])
            nc.sync.dma_start(wv_sb[:], wv_in[:])
            dma_no[0] += 1
            load_xT(1)
            qk0, v0 = proj_pieces(0)
            for p in qk0:
                p()
            carry_v = v0  # v-projection of tile t runs inside tile t's loop
            qk_next, v_next = proj_pieces(1)

            for t in range(NQT):
                nchunks = 4 * (t + 1)
                # stream this tile's E slabs (demand-gated by pool bufs)
                slabs = []
                for g0 in range(0, nchunks, SLAB):
                    gsz = min(SLAB, nchunks - g0)
                    e_slab = epool.tile(
                        [128, SLAB, HPC, QT], F16, tag="e", name="e_slab"
                    )
                    n0 = TILE_IDX[(t, g0)]
                    big_dma(
                        e_slab[:, 0:gsz, :, :],
                        e_in[n0 : n0 + gsz].rearrange("n p h q -> p n h q"),
                    )
                    slabs.append(e_slab)
                if t + 2 < NQT:
                    load_xT(t + 2)
                if t == 4:
                    nc.gpsimd.dma_start(wo_sb[:], wo_in[:])
                    nc.gpsimd.dma_start(bo_sb[:], bo_in[:])
                    nc.gpsimd.dma_start(sel_sb[:], sel_in[:])

                # pieces to interleave: this tile's V projection, then the
                # next tile's Q/K projection
                pieces = carry_v + qk_next
                if t + 1 < NQT:
                    carry_v = v_next
                    if t + 2 < NQT:
                        qk_next, v_next = proj_pieces(t + 2)
                    else:
                        qk_next = []
                npc = [0]

                def emit_proj_piece(n=1):
                    for _ in range(n):
                        if npc[0] < len(pieces):
                            pieces[npc[0]]()
                            npc[0] += 1

                ps_o = [
                    p2o.tile([HD + 1, QT], F32, tag=f"po{h}", name=f"po{h}")
                    for h in range(HPC)
                ]
                nquads = nchunks // QUAD

                def emit_st_exp(t_, c, z_view):
                    ps_s2 = p2s.tile([128, 2 * QT], F32, tag="ps2", name="ps2")
                    for h in range(HPC):
                        nc.tensor.matmul(
                            ps_s2[:, h * QT : (h + 1) * QT],
                            kT_sb[
                                h * HD : (h + 1) * HD,
                                c // 4,
                                (c % 4) * 128 : (c % 4 + 1) * 128,
                            ],
                            qT_sb[h * HD : (h + 1) * HD, t_, :],
                            start=True,
                            stop=True,
                        )
                    nc.scalar.activation(
                        z_view.rearrange("p h q -> p (h q)"),
                        ps_s2[:],
                        mybir.ActivationFunctionType.Exp,
                        bias=nbias[:],
                    )

                def emit_av(pc0, pa, last):
                    for pc in range(QUAD):
                        c = pc0 + pc
                        for h in range(HPC):
                            nc.tensor.matmul(
                                ps_o[h][:],
                                v_sb[h][:, c, :],
                                pa[:, pc, h, :],
                                start=(c == 0),
                                stop=(last and c == nchunks - 1),
                            )

                # AV matmuls lag one pair behind the mul; a 4-matmul AV
                # burst never blocks ACT for longer than one exp
                pending_av = None
                for qd in range(nquads):
                    c0 = qd * QUAD
                    e_slab = slabs[c0 // SLAB]
                    z_sb = zap.tile([128, QUAD, HPC, QT], F16, tag="z", name="z4")
                    a_sb = zap.tile([128, QUAD, HPC, QT], F16, tag="a", name="a4")
                    for pc in range(QUAD):
                        emit_st_exp(t, c0 + pc, z_sb[:, pc, :, :])
                    c_loc = c0 % SLAB
                    nc.vector.tensor_mul(
                        a_sb[:], z_sb[:], e_slab[:, c_loc : c_loc + QUAD, :, :]
                    )
                    emit_proj_piece(-(-len(pieces) // nquads) if pieces else 0)
                    if pending_av is not None:
                        emit_av(pending_av[0], pending_av[1], False)
                    pending_av = (c0, a_sb)
                emit_proj_piece(len(pieces))
                emit_av(pending_av[0], pending_av[1], True)

                ot_sb = otp.tile([HD + 1, HPC, QT], F16, tag="ot")
                for h in range(HPC):
                    nc.vector.tensor_copy(ot_sb[:, h, :], ps_o[h][:])
                # stage this tile's columns into the owning peers' slots
                w = 0 if t < 4 else 1
                wq0 = WQ[w]
                gcol0 = (t - (0 if w == 0 else 4)) * QT
                for p in range(NC):
                    lo = max(gcol0, wq0 * p)
                    hi = min(gcol0 + QT, wq0 * p + wq0)
                    if lo < hi:
                        nc.sync.dma_start(
                            a2a_in[w][p][:, lo - wq0 * p : hi - wq0 * p].rearrange(
                                "(h pp) q -> pp h q", h=HPC
                            ),
                            ot_sb[:, :, lo - gcol0 : hi - gcol0],
                        )
                if phases >= 2.5 and t in (3, 7):
                    w = 0 if t == 3 else 1
                    nc.gpsimd.collective_compute(
                        "AllToAll",
                        mybir.AluOpType.bypass,
                        replica_groups=[list(range(NC))],
                        ins=[a2a_in[w].opt()],
                        outs=[a2a_out[w].opt()],
                    )

            # wave-0 phase-3 loads ride the now-idle gpsimd queue; the
            # reciprocal runs on DVE right after the last multiply
            for l in range(HPC):
                nc.gpsimd.dma_start(
                    den0_sb[l * 8 : (l + 1) * 8, :],
                    a2a_out[0][:, l * (HD + 1) + HD, :],
                )
            nc.vector.reciprocal(rden0_sb[:], den0_sb[:])
            for l in range(HPC):
                nc.gpsimd.dma_start(
                    oT0_sb[l * HD : (l + 1) * HD, :, :],
                    a2a_out[0][:, l * (HD + 1) : l * (HD + 1) + HD, :].rearrange(
                        "c d q -> d c q"
                    ),
                )

            for p in reversed(main_pools):
                p.__exit__(None, None, None)

            # ------------- phase 3: normalize + o_proj (per wave) --------
            # core's output rows: wave 0 -> y_part[0:256], wave 1 ->
            # y_part[256:512]; host maps them to q 256*core (+2048*wave)
            if phases >= 3:
                with (
                    tc.tile_pool(name="p3", bufs=2) as p3,
                    tc.tile_pool(name="p3y", bufs=2) as p3y,
                    tc.tile_pool(name="p3ps", bufs=2, space="PSUM") as p3ps,
                ):
                    for w in range(2):
                        qw = WQ[w]
                        if w == 0:
                            oT_sb, rden_sb = oT0_sb, rden0_sb
                        else:
                            # den row order: l*8 + ci  (head h = 2*ci + l)
                            den_sb = p3.tile([NH, qw], F16, tag="den1", name="den")
                            oT_sb = p3.tile([128, 8, qw], F16, tag="oT1", name="oT")
                            for l in range(HPC):
                                nc.gpsimd.dma_start(
                                    den_sb[l * 8 : (l + 1) * 8, :],
                                    a2a_out[w][:, l * (HD + 1) + HD, :],
                                )
                            rden_sb = p3.tile([NH, qw], F32, tag="rden1", name="rden")
                            nc.vector.reciprocal(rden_sb[:], den_sb[:])
                            for l in range(HPC):
                                nc.gpsimd.dma_start(
                                    oT_sb[l * HD : (l + 1) * HD, :, :],
                                    a2a_out[w][
                                        :, l * (HD + 1) : l * (HD + 1) + HD, :
                                    ].rearrange("c d q -> d c q"),
                                )

                        on_sb = p3.tile([128, 8, qw], BF16, tag=f"on{w}", name="on")
                        for ci in range(8):
                            ps_b = p3ps.tile([128, qw], F32, tag="bc", name="bc")
                            nc.tensor.matmul(
                                ps_b[:],
                                sel_sb[:, ci * 128 : (ci + 1) * 128],
                                rden_sb[:],
                                start=True,
                                stop=True,
                            )
                            nc.vector.tensor_mul(
                                on_sb[:, ci, :], oT_sb[:, ci, :], ps_b[:]
                            )

                        for qb in range(qw // 128):
                            y_sb = p3y.tile([128, H], F32, tag="y", name="y")
                            for nh2 in range(2):
                                ps_y = p3ps.tile([128, QT], F32, tag="py", name="py")
                                for c in range(8):
                                    nc.tensor.matmul(
                                        ps_y[:],
                                        on_sb[:, c, qb * 128 : (qb + 1) * 128],
                                        wo_sb[:, c, nh2 * QT : (nh2 + 1) * QT],
                                        start=(c == 0),
                                        stop=(c == 7),
                                    )
                                nc.vector.tensor_add(
                                    y_sb[:, nh2 * QT : (nh2 + 1) * QT],
                                    ps_y[:],
                                    bo_sb[:, nh2 * QT : (nh2 + 1) * QT],
                                )
                            nc.sync.dma_start(
                                y_out[
                                    w * WQ[0] + qb * 128 : w * WQ[0]
                                    + (qb + 1) * 128,
                                    :,
                                ],
                                y_sb[:],
                            )

            if phases < 3:
                with tc.tile_pool(name="dbg", bufs=1) as dbgp:
                    dbg = dbgp.tile([128, 2, QT], F32, tag="dbg")
                    nc.vector.tensor_copy(dbg[:], qT_sb[:, 0:2, :].bitcast(F32))
                    nc.sync.dma_start(
                        y_out[0:128, :], dbg[:].rearrange("p a b -> p (a b)")
                    )

    nc.compile()
    return nc


_PROGRAM_CACHE = {}


def _get_program():
    if "nc" not in _PROGRAM_CACHE:
        _PROGRAM_CACHE["nc"] = _build_program()
    return _PROGRAM_CACHE["nc"]


def _host_prep(x, idx, valid, geo_bias, Wq, Wk, Wv, Wo, bo):
    x2 = np.ascontiguousarray(np.asarray(x, dtype=np.float32).reshape(S, H))
    idx = np.asarray(idx).astype(np.int64)
    valid = np.asarray(valid).astype(bool)
    geo = np.asarray(geo_bias, dtype=np.float32)
    Wq = np.asarray(Wq, dtype=np.float32)
    Wk = np.asarray(Wk, dtype=np.float32)
    Wv = np.asarray(Wv, dtype=np.float32)
    Wo = np.asarray(Wo, dtype=np.float32)
    bo = np.asarray(bo, dtype=np.float32)

    qpos = np.arange(S, dtype=np.int64)[:, None]
    keep = valid & (idx <= qpos) & (idx >= 0)
    s_flat = idx[keep]
    q_flat = np.broadcast_to(qpos, idx.shape)[keep]
    lin = s_flat * S + q_flat

    bo_rep = np.ascontiguousarray(np.broadcast_to(bo[None, :], (128, H)))

    # den row order in phase 3 is r = l*8 + ci for head h = 2*ci + l
    sel16 = np.zeros((NH, H), dtype=np.float32)
    ch = np.arange(H)
    sel16[((ch // HD) % 2) * 8 + ch // 128, ch] = 1.0

    wq_scaled = Wq / np.sqrt(HD)

    # x pre-transposed: [st, p, hc, 512] with value x[st*512+s, hc*128+p]
    xT = np.ascontiguousarray(
        x2.T.reshape(8, 128, NQT, QT).transpose(2, 1, 0, 3)
    ).astype(ml_dtypes.bfloat16)

    def wslice(W, core):
        cs = slice(128 * core, 128 * (core + 1))
        return np.ascontiguousarray(
            W[:, cs].reshape(8, 128, 128).transpose(1, 0, 2)
        ).astype(ml_dtypes.bfloat16)

    wo_t = np.ascontiguousarray(Wo.reshape(8, 128, H).transpose(1, 0, 2)).astype(
        ml_dtypes.bfloat16
    )

    in_maps = []
    for core in range(NC):
        e_pack = np.empty((N_TILES, SC, HPC, QT), dtype=np.float16)
        for l in range(HPC):
            h = HPC * core + l
            w = np.exp(geo[h][keep].astype(np.float64))
            eT = np.bincount(lin, weights=w, minlength=S * S).reshape(S, S)
            for n, (t, c) in enumerate(TILE_LIST):
                e_pack[n, :, l, :] = eT[
                    c * SC : (c + 1) * SC, t * QT : (t + 1) * QT
                ].astype(np.float16)
        in_maps.append(
            {
                "xT": xT,
                "wq": wslice(wq_scaled, core),
                "wk": wslice(Wk, core),
                "wv": wslice(Wv, core),
                "wo": wo_t,
                "bo_rep": bo_rep,
                "e_pack": e_pack,
                "sel16": sel16,
            }
        )
    return in_maps


LAST_RESULTS = None


def kernel(x, idx, valid, geo_bias, Wq, Wk, Wv, Wo, bo):
    global LAST_RESULTS
    b, s, h = np.asarray(x).shape
    assert (b, s, h) == (1, S, H)
    in_maps = _host_prep(x, idx, valid, geo_bias, Wq, Wk, Wv, Wo, bo)
    nc = _get_program()
    res = run_bass_kernel_spmd(nc, in_maps, core_ids=list(range(NC)))
    LAST_RESULTS = res
    y = np.empty((S, H), dtype=np.float32)
    for c in range(NC):
        yp = np.asarray(res.results[c]["y_part"], dtype=np.float32)
        y[256 * c : 256 * c + 256] = yp[0:256]
        y[2048 + 256 * c : 2048 + 256 * c + 256] = yp[256:512]
    return y.reshape(1, S, H).astype(np.float32)
